# revision 10
# baseline (speedup 1.0000x reference)
"""Trainium2 Bass kernel for nn_MultiHead_32006096290468.

LayerNorm(Q/K/V) -> 4-head attention (d_head=8) with multiplicative |mask|
applied to pre-softmax scores -> output projection.  B=32 sharded 4-per-core
across 8 NeuronCores (pure data parallel); all params replicated.

Per-core dataflow (per batch b):
  - DMA Q,K,V [1024,256] and mask [1024,1024] (natural layout).
  - LayerNorm: bn_stats/bn_aggr per 128-token tile -> (mu, var);
    rs = exp(-0.5*ln(var+eps)) on ACT (Ln/Exp share one table set);
    in-place (x-mu)*rs via a single two-scalar tensor_scalar.
  - PE-transpose qn -> qnT [256,1024]; project with ln/scale-folded weights
    into a "spread" channel layout (head h channels at partitions 32h..32h+7)
    so the 4 heads' K=8 QK^T matmuls row-tile into disjoint PE row groups.
  - scoresT[k,q] = kT_h-slice.T @ qT_h  (PSUM), then a single fused DVE
    scalar_tensor_tensor: masked = |maskT| * scoresT  (abs folded at the
    maskT PSUM->SBUF evacuation), exp on ACT (no max-subtraction needed:
    scores are O(1) by construction), and PV matmuls col-tiled per head with
    a ones-column folded in to produce softmax denominators for free:
      pv[32h+0]   = rowsum_h,  pv[32h+1..9] = xT_h,  rest 0.
  - xTs = pv * broadcast(1/rowsum); out = xTs.T @ Wo_spread (+nothing: all
    biases are exactly zero but q/k/v biases are still applied per-partition
    during the PSUM evacuations).
mask is transposed just-in-time per key-tile on the PE (scoresT layout needs
maskT; transposing the 4x smaller mask instead of exp-scores).
"""
import sys

sys.path.insert(0, "/opt/trn_rl_repo")

import numpy as np

B, L, D, C, H, DH, E = 32, 1024, 256, 32, 4, 8, 256
NCORES = 8
BPC = B // NCORES
LT = L // 128
EPS = 1e-5

_cache = {}


def _build_nc(bpc, nrep=1):
    import concourse.bacc as bacc
    import concourse.tile as tile
    import concourse.mybir as mybir
    from concourse.masks import make_identity

    F32 = mybir.dt.float32
    AF = mybir.ActivationFunctionType
    OP = mybir.AluOpType

    nc = bacc.Bacc("TRN2", target_bir_lowering=False, debug=False,
                   num_devices=NCORES)

    q_in = nc.dram_tensor("q_in", [bpc, L, D], F32, kind="ExternalInput").ap()
    k_in = nc.dram_tensor("k_in", [bpc, L, D], F32, kind="ExternalInput").ap()
    v_in = nc.dram_tensor("v_in", [bpc, L, D], F32, kind="ExternalInput").ap()
    m_in = nc.dram_tensor("m_in", [bpc, L, L], F32, kind="ExternalInput").ap()
    wq_d = nc.dram_tensor("wq", [D, 128], F32, kind="ExternalInput").ap()
    wk_d = nc.dram_tensor("wk", [D, 128], F32, kind="ExternalInput").ap()
    wv_d = nc.dram_tensor("wv", [D, C], F32, kind="ExternalInput").ap()
    wo_d = nc.dram_tensor("wo", [128, E], F32, kind="ExternalInput").ap()
    bq_d = nc.dram_tensor("bq", [128, 1], F32, kind="ExternalInput").ap()
    bk_d = nc.dram_tensor("bk", [128, 1], F32, kind="ExternalInput").ap()
    bv_d = nc.dram_tensor("bv", [C, 1], F32, kind="ExternalInput").ap()
    out_d = nc.dram_tensor("out", [bpc, L, E], F32, kind="ExternalOutput").ap()

    with tile.TileContext(nc) as tc:
        with (
            tc.tile_pool(name="consts", bufs=1) as consts,
            tc.tile_pool(name="qkvin", bufs=1) as qkvin,
            tc.tile_pool(name="masknat", bufs=2) as masknat,
            tc.tile_pool(name="lnp", bufs=2) as lnp,
            tc.tile_pool(name="qntp", bufs=2) as qntp,
            tc.tile_pool(name="proj", bufs=2) as projp,
            tc.tile_pool(name="mtp", bufs=2) as mtp,
            tc.tile_pool(name="mskd", bufs=2) as mskdp,
            tc.tile_pool(name="expp", bufs=2) as expp,
            tc.tile_pool(name="post", bufs=1) as postp,
            tc.tile_pool(name="outp", bufs=1) as outp,
            tc.tile_pool(name="scps", bufs=2, space="PSUM") as scps,
            tc.tile_pool(name="pvps", bufs=1, space="PSUM") as pvps,
            tc.tile_pool(name="mips", bufs=2, space="PSUM") as mips,
        ):
            ident = consts.tile([128, 128], F32)
            make_identity(nc, ident)
            eps_t = consts.tile([128, 1], F32)
            nc.vector.memset(eps_t, EPS)
            ones_t = consts.tile([128, C], F32)
            nc.vector.memset(ones_t, 1.0)

            wq_sb = consts.tile([128, 2, 128], F32)
            wk_sb = consts.tile([128, 2, 128], F32)
            wv_sb = consts.tile([128, 2, C], F32)
            wo_sb = consts.tile([128, E], F32)
            nc.sync.dma_start(out=wq_sb, in_=wq_d.rearrange("(c p) m -> p c m", p=128))
            nc.sync.dma_start(out=wk_sb, in_=wk_d.rearrange("(c p) m -> p c m", p=128))
            nc.sync.dma_start(out=wv_sb, in_=wv_d.rearrange("(c p) m -> p c m", p=128))
            nc.sync.dma_start(out=wo_sb, in_=wo_d)
            bq_sb = consts.tile([128, 1], F32)
            bk_sb = consts.tile([128, 1], F32)
            bv_sb = consts.tile([C, 1], F32)
            nc.sync.dma_start(out=bq_sb, in_=bq_d)
            nc.sync.dma_start(out=bk_sb, in_=bk_d)
            nc.sync.dma_start(out=bv_sb, in_=bv_d)

            for b in [bb for _ in range(nrep) for bb in range(bpc)]:
                qn = qkvin.tile([128, LT, D], F32, tag="qn")
                kn = qkvin.tile([128, LT, D], F32, tag="kn")
                vn = qkvin.tile([128, LT, D], F32, tag="vn")
                msk = masknat.tile([128, LT, L], F32, tag="msk")
                nc.sync.dma_start(out=qn, in_=q_in[b].rearrange("(t p) d -> p t d", p=128))
                nc.sync.dma_start(out=kn, in_=k_in[b].rearrange("(t p) d -> p t d", p=128))
                nc.sync.dma_start(out=vn, in_=v_in[b].rearrange("(t p) d -> p t d", p=128))
                nc.sync.dma_start(out=msk, in_=m_in[b].rearrange("(t p) k -> p t k", p=128))

                # ---- LayerNorm (in place) + transpose + project ----
                qT = projp.tile([128, L], F32, tag="qT")
                kT = projp.tile([128, L], F32, tag="kT")
                vT = projp.tile([C, L], F32, tag="vT")
                for x_t, w_sb, b_sb, xT, wid in (
                    (qn, wq_sb, bq_sb, qT, 128),
                    (kn, wk_sb, bk_sb, kT, 128),
                    (vn, wv_sb, bv_sb, vT, C),
                ):
                    st = lnp.tile([128, LT, 6], F32, tag="st")
                    mv = lnp.tile([128, LT, 2], F32, tag="mv")
                    for t in range(LT):
                        nc.vector.bn_stats(st[:, t], x_t[:, t])
                        nc.vector.bn_aggr(mv[:, t], st[:, t])
                    lnv = lnp.tile([128, LT], F32, tag="lnv")
                    rs = lnp.tile([128, LT], F32, tag="rs")
                    nc.scalar.activation(lnv, mv[:, :, 1:2], AF.Ln, bias=eps_t)
                    nc.scalar.activation(rs, lnv, AF.Exp, scale=-0.5)
                    for t in range(LT):
                        nc.vector.tensor_scalar(
                            out=x_t[:, t], in0=x_t[:, t],
                            scalar1=mv[:, t, 0:1], scalar2=rs[:, t:t + 1],
                            op0=OP.subtract, op1=OP.mult)
                    # transpose normalized activations -> xnT [2,128 x 1024]
                    xnT = qntp.tile([128, 2, L], F32, tag="xnT")
                    for dc in range(2):
                        for tg in range(2):
                            tp = mips.tile([128, 512], F32, tag="m")
                            for j in range(4):
                                t = tg * 4 + j
                                nc.tensor.transpose(
                                    tp[:, j * 128:(j + 1) * 128],
                                    x_t[:, t, dc * 128:(dc + 1) * 128], ident)
                            nc.vector.tensor_copy(
                                xnT[:, dc, tg * 512:(tg + 1) * 512], tp)
                    # project: xT[sp(c), l] = sum_d W[d, sp(c)] xnT[d, l] (+bias)
                    for nchk in range(2):
                        pp = mips.tile([128, 512], F32, tag="m")
                        for dc in range(2):
                            nc.tensor.matmul(
                                pp[0:wid, :], w_sb[:, dc, 0:wid],
                                xnT[:, dc, nchk * 512:(nchk + 1) * 512],
                                start=(dc == 0), stop=(dc == 1))
                        nc.scalar.activation(
                            xT[:, nchk * 512:(nchk + 1) * 512], pp[0:wid, :],
                            AF.Identity, bias=b_sb)

                # ---- v_aug: [128keys, 128] per key tile; head h columns:
                #      32h+0 = ones, 32h+1..9 = v channels, rest zeros ----
                vaug = projp.tile([128, LT, 128], F32, tag="vaug")
                nc.vector.memset(vaug, 0.0)
                nc.vector.memset(
                    vaug.rearrange("p t (h x) -> p t h x", x=32)[:, :, :, 0:1], 1.0)
                for tg in range(2):
                    tp = mips.tile([128, 512], F32, tag="m")
                    for j in range(4):
                        kt = tg * 4 + j
                        nc.tensor.transpose(
                            tp[:, j * C:(j + 1) * C],
                            vT[:, kt * 128:(kt + 1) * 128], ident[0:C, 0:C])
                    # scatter channels c=8h+cc to column 32h+1+cc of tile kt
                    nc.vector.tensor_copy(
                        vaug.rearrange("p t (h x) -> p t h x", x=32)
                            [:, tg * 4:tg * 4 + 4, :, 1:9],
                        tp[:, 0:4 * C].rearrange("p (j h c) -> p j h c", j=4, h=H))
                # ---- attention ----
                pv = pvps.tile([128, L], F32, tag="pv")
                for kt in range(LT):
                    mt = mtp.tile([128, L], F32, tag="mt")
                    for tg in range(2):
                        tp = mips.tile([128, 512], F32, tag="m")
                        for j in range(4):
                            qt = tg * 4 + j
                            nc.tensor.transpose(
                                tp[:, j * 128:(j + 1) * 128],
                                msk[:, qt, kt * 128:(kt + 1) * 128], ident)
                        nc.scalar.activation(
                            mt[:, tg * 512:(tg + 1) * 512], tp, AF.Abs)
                    for hp in range(2):
                        mskd = mskdp.tile([128, 2048], F32, tag="mskd")
                        ex = expp.tile([128, 2048], F32, tag="ex")
                        for hh in range(2):
                            h = hp * 2 + hh
                            sc = scps.tile([128, L], F32, tag="sc")
                            for nchk in range(2):
                                nc.tensor.matmul(
                                    sc[:, nchk * 512:(nchk + 1) * 512],
                                    kT[32 * h:32 * h + DH, kt * 128:(kt + 1) * 128],
                                    qT[32 * h:32 * h + DH, nchk * 512:(nchk + 1) * 512],
                                    start=True, stop=True,
                                    tile_position=(32 * h, 0))
                            nc.vector.scalar_tensor_tensor(
                                out=mskd[:, hh * 1024:(hh + 1) * 1024],
                                in0=mt, scalar=1.0, in1=sc,
                                op0=OP.mult, op1=OP.mult)
                        nc.scalar.activation(ex, mskd, AF.Exp)
                        for hh in range(2):
                            h = hp * 2 + hh
                            for nchk in range(2):
                                nc.tensor.matmul(
                                    pv[32 * h:32 * h + 32, nchk * 512:(nchk + 1) * 512],
                                    vaug[:, kt, 32 * h:32 * h + 32],
                                    ex[:, hh * 1024 + nchk * 512: hh * 1024 + (nchk + 1) * 512],
                                    start=(kt == 0), stop=(kt == LT - 1),
                                    skip_group_check=True,
                                    tile_position=(0, 32 * h))
                # ---- normalize + output projection ----
                rb = postp.tile([128, L], F32, tag="rb")
                nc.vector.reciprocal(rb, pv)
                rcb = postp.tile([128, L], F32, tag="rcb")
                for nchk in range(2):
                    rp = mips.tile([128, 512], F32, tag="m")
                    for h in range(H):
                        nc.tensor.matmul(
                            rp[32 * h:32 * h + 32, :],
                            ones_t[32 * h:32 * h + 1, :],
                            rb[32 * h:32 * h + 1, nchk * 512:(nchk + 1) * 512],
                            start=True, stop=True,
                            tile_position=(32 * h, 32 * h))
                    nc.scalar.copy(rcb[:, nchk * 512:(nchk + 1) * 512], rp)
                xts = postp.tile([128, L], F32, tag="xts")
                nc.vector.tensor_mul(xts, pv, rcb)
                outb = outp.tile([128, LT, E], F32, tag="outb")
                for og in range(LT // 2):
                    op_ps = mips.tile([128, 512], F32, tag="m")
                    for j in range(2):
                        lc = og * 2 + j
                        nc.tensor.matmul(
                            op_ps[:, j * E:(j + 1) * E],
                            xts[:, lc * 128:(lc + 1) * 128], wo_sb,
                            start=True, stop=True)
                    nc.scalar.copy(
                        outb.rearrange("p t e -> p (t e)")[:, og * 512:(og + 1) * 512],
                        op_ps)
                nc.sync.dma_start(
                    out=out_d[b].rearrange("(t p) e -> p t e", p=128), in_=outb)

    nc.compile()
    return nc


def _host_params(inputs):
    g = np.asarray(inputs["ln_g"], np.float32)
    bta = np.asarray(inputs["ln_b"], np.float32)
    W_q = np.asarray(inputs["W_q"], np.float32)
    W_k = np.asarray(inputs["W_k"], np.float32)
    W_v = np.asarray(inputs["W_v"], np.float32)
    W_o = np.asarray(inputs["W_o"], np.float32)
    b_q = np.asarray(inputs["b_q"], np.float32)
    b_k = np.asarray(inputs["b_k"], np.float32)
    b_v = np.asarray(inputs["b_v"], np.float32)
    b_o = np.asarray(inputs["b_o"], np.float32)
    s = 1.0 / np.sqrt(DH)
    Wq_e = (g[:, None] * W_q) * s
    Wk_e = g[:, None] * W_k
    Wv_e = g[:, None] * W_v
    bq_e = (bta @ W_q + b_q) * s
    bk_e = bta @ W_k + b_k
    bv_e = bta @ W_v + b_v
    assert np.abs(b_o).max() == 0.0, "nonzero b_o not folded"

    def spread_qk(w):  # [D, 32] -> [D, 128], channel 8h+cc at col 32h+cc
        out = np.zeros((D, 128), np.float32)
        for c in range(C):
            out[:, 32 * (c // 8) + (c % 8)] = w[:, c]
        return out

    wq_sp = spread_qk(Wq_e)
    wk_sp = spread_qk(Wk_e)
    bq_sp = np.zeros((128, 1), np.float32)
    bk_sp = np.zeros((128, 1), np.float32)
    for c in range(C):
        bq_sp[32 * (c // 8) + (c % 8), 0] = bq_e[c]
        bk_sp[32 * (c // 8) + (c % 8), 0] = bk_e[c]
    wo_sp = np.zeros((128, E), np.float32)
    for c in range(C):
        wo_sp[32 * (c // 8) + 1 + (c % 8)] = W_o[c]
    return dict(wq=wq_sp, wk=wk_sp, wv=Wv_e, wo=wo_sp,
                bq=bq_sp, bk=bk_sp, bv=bv_e.reshape(C, 1))


def kernel(**inputs):
    from concourse.bass_utils import run_bass_kernel_spmd
    if "nc" not in _cache:
        _cache["nc"] = _build_nc(BPC)
    nc = _cache["nc"]
    params = _host_params(inputs)
    Q = np.asarray(inputs["Q"], np.float32)
    K = np.asarray(inputs["K"], np.float32)
    V = np.asarray(inputs["V"], np.float32)
    M = np.asarray(inputs["mask"], np.float32)
    in_maps = []
    for i in range(NCORES):
        sl = slice(i * BPC, (i + 1) * BPC)
        in_maps.append(dict(q_in=Q[sl], k_in=K[sl], v_in=V[sl], m_in=M[sl],
                            **params))
    res = run_bass_kernel_spmd(nc, in_maps, list(range(NCORES)))
    return np.concatenate([res.results[i]["out"] for i in range(NCORES)], 0)
